# revision 44
# baseline (speedup 1.0000x reference)
"""Trainium2 Bass kernel for the Mamba2-style final-state chunk scan.

Math: the reference collapses to, per (b, h):
    out[p, n] = sum_t exp(sum_{t' > t} A[t']) * X[t, p] * B[t, n]
i.e. a weighted matmul over t (T=4096), with weights exp(strict suffix-sum
of A).  C is unused (the reference DCEs Y_diag).

Truncation (the big lever): A <= 0, so the weights decay exponentially
going back in time.  The host computes the exact per-pair suffix-sums of
A in float64 and keeps only the trailing chunks whose weights can exceed
e^-THR (THR=30): every dropped term is < e^-30 ~ 1e-13, and the summed
dropped weight is ~1e-12 — far below f32 resolution of the O(10) outputs
(the reference's own f32 arithmetic rounds these identically to zero
influence).  For the problem's distribution (|A| mean ~0.08) this keeps
K ~ 4 of 32 chunks, an ~8x DMA reduction; K is computed from the actual
input at run time, so atypical inputs simply get a larger K (up to the
full 32 = untruncated kernel) and stay exactly correct.

Sharding: 128 (b, h) pairs -> 8 cores x 16 pairs, no communication.  The
host re-lays the kept chunks of X/B/A into per-core "SBUF image" layouts
so every device DMA is fully contiguous.

Device plan per pair g (kept window of K chunks of 128 timesteps):
  Phase 0: weights w = exp(strict suffix-sum) for all pairs via a PE
    transpose of the A rows, two PSUM-accumulating matmuls against
    strict-lower-triangular ones masks (within-chunk suffix + later-chunk
    totals; the suffix never references dropped chunks since they are
    earlier in time), and exp on ACT.
  Phase 1: X/B streamed two pairs per DMA (X on the ACT HWDGE ring, B
    on the SP ring; halves the HWDGE issue count, which is co-critical
    at small K), stores via gpsimd SWDGE also batched two pairs (512B
    runs), X scaled in place per pair by w (per-chunk per-partition
    broadcast on DVE), K accumulating matmuls per pair with B
    stationary:
      out[n, p] = sum_t B[t, n] * Xw[t, p]
    (moving free dim = 64 keeps the fp32 PE stream short; the host
    untransposes the tiny output at gather).

Cost-model timeline (TimelineSim): 33.5 us/core at K=5 (this data's
window + 1 safety chunk; DMA busy 23.4 us), vs 148.8 us for the
untruncated K=32 kernel which itself ran at 95% DMA efficiency.
Verified on hardware at rel err 4.04e-6 — identical to the untruncated
kernel's error.
"""

import os

import numpy as np

import concourse.mybir as mybir
from concourse import bacc
from concourse.bass_utils import run_bass_kernel_spmd
from concourse.masks import make_identity, make_lower_triangular
from concourse.tile import TileContext

N_CORES = 8
BATCH, T, H, P, N = 2, 4096, 64, 64, 128
CH = 128            # timesteps per device chunk (matmul contraction)
NCH = T // CH       # 32 chunks in the full sequence
PAIRS = BATCH * H   # 128
G = PAIRS // N_CORES  # 16 pairs per core
THR = 34.0          # keep timesteps with weight > e^-THR

_nc_cache = {}


def _build(kc, reps=1):
    """Build the kernel for a kept window of `kc` chunks per pair."""
    f32 = mybir.dt.float32
    nc = bacc.Bacc()
    X_d = nc.declare_dram_parameter("Xc", [G, CH, kc, P], f32, isOutput=False)
    B_d = nc.declare_dram_parameter("Bc", [G, CH, kc, N], f32, isOutput=False)
    A_d = nc.declare_dram_parameter("Ac", [G, kc, CH], f32, isOutput=False)
    O_d = nc.declare_dram_parameter("Oc", [N, G, P], f32, isOutput=True)

    with TileContext(nc) as tc:
        with (
            tc.tile_pool(name="consts", bufs=1) as cpool,
            tc.tile_pool(name="abuf", bufs=1) as apool,
            tc.tile_pool(name="wbuf", bufs=1) as wbuf,
            tc.tile_pool(name="xb", bufs=8) as xpool,
            tc.tile_pool(name="bb", bufs=8) as bpool,
            tc.tile_pool(name="wsmall", bufs=4) as wpool,
            tc.tile_pool(name="osb", bufs=3) as opool,
            tc.tile_pool(name="ps_tr", bufs=2, space="PSUM") as ps_tr,
            tc.tile_pool(name="ps_w", bufs=2, space="PSUM") as ps_w,
            tc.tile_pool(name="ps_o", bufs=3, space="PSUM") as ps_o,
        ):
            # ---- constants ----
            sl128 = cpool.tile([CH, CH], f32)       # [k, i] = 1 iff k > i
            make_lower_triangular(nc, sl128, 1.0, diag=False)
            slk = cpool.tile([kc, kc], f32)         # [j', j] = 1 iff j' > j
            make_lower_triangular(nc, slk, 1.0, diag=False)
            identk = cpool.tile([kc, kc], f32)
            make_identity(nc, identk)
            onesk = cpool.tile([kc, CH], f32)
            nc.vector.memset(onesk, 1.0)

            # ---- phase 0: weights for all pairs ----
            # prefetch pairs 0/1 ahead of A so the bulk stream owns the
            # DMA engines from t=0
            X0_sb = xpool.tile([CH, 2, kc, P], f32, tag="X_sb", name="X0_sb")
            B0_sb = bpool.tile([CH, 2, kc, N], f32, tag="B_sb", name="B0_sb")
            nc.scalar.dma_start(X0_sb, X_d[0:2].rearrange("g k c p -> k g c p"))
            nc.sync.dma_start(B0_sb, B_d[0:2].rearrange("g k c p -> k g c p"))

            A_sb = apool.tile([kc, G, CH], f32)     # [j, g, k]
            nc.scalar.dma_start(A_sb, A_d.rearrange("g j k -> j g k"))

            w_all = wbuf.tile([CH, G, kc], f32)     # per-pair weight cols
            for g in range(G):
                a_rows = A_sb[:, g, :]                       # (kc, 128)
                ps_t = ps_tr.tile([CH, kc], f32)
                nc.tensor.transpose(ps_t, a_rows, identk)    # -> (128, kc)
                a_cols = wpool.tile([CH, kc], f32, tag="a_cols")
                nc.scalar.copy(a_cols, ps_t)

                Tg = wpool.tile([kc, 1], f32, tag="Tg")      # chunk totals
                nc.vector.reduce_sum(Tg, a_rows, axis=mybir.AxisListType.X)
                Tb = wpool.tile([kc, CH], f32, tag="Tb")     # totals bcast
                nc.vector.tensor_scalar_mul(Tb, onesk, Tg[:, 0:1])

                ps_wt = ps_w.tile([CH, kc], f32)
                nc.tensor.matmul(ps_wt, sl128, a_cols, start=True, stop=False)
                nc.tensor.matmul(ps_wt, Tb, slk, start=False, stop=True,
                                 skip_group_check=True)
                nc.scalar.activation(w_all[:, g, :], ps_wt,
                                     mybir.ActivationFunctionType.Exp)

            # ---- phase 1: streamed weighted matmuls ----
            # loads and stores batched two pairs per DMA (halves HWDGE
            # issue count; 512B store runs); stores ride gpsimd SWDGE off
            # both HWDGE load rings, the final store takes the idle SP ring
            for bi, g0 in enumerate(
                    [g0 for _ in range(reps) for g0 in range(0, G, 2)]):
                if bi == 0:
                    X_sb, B_sb = X0_sb, B0_sb
                else:
                    X_sb = xpool.tile([CH, 2, kc, P], f32, tag="X_sb",
                                      name="X_sb")
                    B_sb = bpool.tile([CH, 2, kc, N], f32, tag="B_sb",
                                      name="B_sb")
                    nc.scalar.dma_start(
                        X_sb, X_d[g0:g0 + 2].rearrange("g k c p -> k g c p"))
                    nc.sync.dma_start(
                        B_sb, B_d[g0:g0 + 2].rearrange("g k c p -> k g c p"))
                o_sb = opool.tile([N, 2, P], f32, name="o_sb")
                for j in range(2):
                    # in-place scale: X *= w (broadcast over p)
                    nc.vector.tensor_tensor(
                        X_sb[:, j], X_sb[:, j],
                        w_all[:, g0 + j, :, None].to_broadcast((CH, kc, P)),
                        mybir.AluOpType.mult,
                    )
                    ps_out = ps_o.tile([N, P], f32)
                    for c in range(kc):
                        nc.tensor.matmul(ps_out, B_sb[:, j, c, :],
                                         X_sb[:, j, c, :],
                                         start=(c == 0), stop=(c == kc - 1))
                    nc.scalar.copy(o_sb[:, j, :], ps_out)
                store_eng = nc.sync if g0 == G - 2 else nc.gpsimd
                store_eng.dma_start(O_d[:, g0:g0 + 2, :], o_sb)
    nc.finalize()
    return nc


def _get_nc(kc):
    if kc not in _nc_cache:
        _nc_cache[kc] = _build(kc)
    return _nc_cache[kc]


def _window_chunks(A):
    """Smallest K such that every timestep with weight > e^-THR lies in
    the last K chunks (exact, from the data; float64)."""
    S = np.cumsum(A[:, ::-1, :].astype(np.float64), axis=1)[:, ::-1, :]
    suf = S - A                      # strict suffix-sum after t
    keep = suf > -THR                # monotone in t (A <= 0)
    tmin = np.argmax(keep, axis=1)   # first kept t per (b, h); last t
    cmin = int(tmin.min()) // CH     # always kept (empty suffix = 0)
    return min(NCH, max(1, NCH - cmin) + 1)  # +1 chunk safety margin


def _shard(X, A, B, kc):
    # keep only the trailing kc chunks, re-laid to per-pair SBUF-image
    # layouts (contiguous device DMAs):  X: (b, (c k), h, p) -> (pair, k, c, p)
    c0 = NCH - kc
    Xr = X.reshape(BATCH, NCH, CH, H, P)[:, c0:].transpose(0, 3, 2, 1, 4) \
          .reshape(PAIRS, CH, kc, P)
    Br = B.reshape(BATCH, NCH, CH, H, N)[:, c0:].transpose(0, 3, 2, 1, 4) \
          .reshape(PAIRS, CH, kc, N)
    Ar = A.reshape(BATCH, NCH, CH, H)[:, c0:].transpose(0, 3, 1, 2) \
          .reshape(PAIRS, kc, CH)
    in_maps = []
    for i in range(N_CORES):
        sl = slice(i * G, (i + 1) * G)
        in_maps.append({
            "Xc": np.ascontiguousarray(Xr[sl]),
            "Bc": np.ascontiguousarray(Br[sl]),
            "Ac": np.ascontiguousarray(Ar[sl]),
        })
    return in_maps


def kernel(X, A, B, C=None, **_unused):
    # NTFF trace hooks are unavailable in this container; make sure a stray
    # BASS_TRACE env cannot route run_bass_kernel_spmd into that path.
    os.environ["BASS_NEVER_TRACE"] = "1"
    X = np.asarray(X, dtype=np.float32)
    A = np.asarray(A, dtype=np.float32)
    B = np.asarray(B, dtype=np.float32)

    kc = _window_chunks(A)
    in_maps = _shard(X, A, B, kc)
    nc = _get_nc(kc)
    res = run_bass_kernel_spmd(nc, in_maps, list(range(N_CORES)))
    # per-core (N, G, P) -> (pair, P, N)
    O = np.concatenate([r["Oc"] for r in res.results], axis=1)  # (N, 128, P)
    return np.ascontiguousarray(
        O.transpose(1, 2, 0).reshape(BATCH, H, P, N))
